# revision 6
# baseline (speedup 1.0000x reference)
"""DSH loss kernel for Trainium2 (8 NeuronCores, Bass/Tile).

Math (reference):
    U[ind] = u; Y[ind] = y
    raw[b,n]  = ||u_b||^2 - 2 u_b.U_n + ||U_n||^2          (>= 0 mathematically)
    dist      = max(raw, 0)
    match[b,n]= y_b . Y_n          (integer >= 0)
    m         = (match == 0)       ("mismatch" mask, statistically ~never 1)
    loss1 = mean( (1-m)*0.5*dist + m*0.5*relu(M - dist) )
    loss2 = ALPHA * mean(|1 - sign(u)|)

Decomposition used here:
    2*B*N*loss1 = S_raw + S_hinge - S_mraw
      S_raw   = sum_{b,n} raw          -> exact on host in fp64 (factorizes:
                sum raw = N*sum(u_sq) + B*sum(U_sq) - 2*colsum(u).colsum(U))
      S_hinge = sum_{m=1} relu(M - raw) -> on device: relu(M - x1) where
                x1 = raw + BIG*match accumulated in one PSUM group (two
                matmuls); match>=1 makes M - x1 hugely negative -> exact 0.
                ScalarE activation computes relu with fused accum_out.
      S_mraw  = sum_{m=1} raw          -> host, using the device's per
                (row, n-tile) min(x1) detector: min < BIG/2 iff some
                match==0 in that row-tile. Flagged row-tiles (normally
                none) are recomputed exactly on host in fp64.

Device per core (shard = 12500 columns of U/Y):
    for b-chunk (4 x 128) x n-tile (512 wide):
        PSUM x1 = uaT_chunk.T @ UA_tile + (BIG*y)_chunk.T @ Y_tile
        ACT: relu(M - x1) with accum_out -> accA column
        DVE: min-reduce x1 -> accMin column
"""

import os
import numpy as np

import concourse.bass as bass
import concourse.mybir as mybir
import concourse.tile as tile
from concourse import bacc
from concourse.bass_utils import run_bass_kernel_spmd

# Problem constants (hardcoded per harness contract)
B = 512
BIT = 64
C = 100
N = 100000
N_CORES = 8
N_SH = N // N_CORES          # 12500
M_MARGIN = 2.0 * BIT         # 128.0
ALPHA = 0.1
BIG = 16384.0                # power of two; BIG*match exact in fp32
DET_THRESH = BIG / 2.0       # min(x1) < this  <=>  some match==0 in row-tile

F_TILE = 512                 # matmul moving free dim / PSUM bank width (fp32)
KA = BIT + 2                 # augmented contraction dim for the dist matmul
N_CHUNKS = B // 128          # 4


def _build_program(n_sh: int, dt_in):
    """Build the per-core Bass/Tile program. Returns (nc, n_tiles)."""
    fp32 = mybir.dt.float32
    nc = bacc.Bacc("TRN2", target_bir_lowering=False)

    n_tiles = (n_sh + F_TILE - 1) // F_TILE
    n_iters = N_CHUNKS * n_tiles

    # External I/O
    uaT_d = nc.declare_dram_parameter("uaT", [KA, B], dt_in, isOutput=False)
    ypT_d = nc.declare_dram_parameter("ypT", [C, B], dt_in, isOutput=False)
    UA_d = nc.declare_dram_parameter("UA", [KA, n_sh], dt_in, isOutput=False)
    YT_d = nc.declare_dram_parameter("YT", [C, n_sh], dt_in, isOutput=False)
    accA_d = nc.declare_dram_parameter("accA", [128, n_iters], fp32, isOutput=True)
    accMin_d = nc.declare_dram_parameter("accMin", [128, n_iters], fp32, isOutput=True)

    with tile.TileContext(nc) as tc:
        with (
            tc.tile_pool(name="resident", bufs=1) as resident,
            tc.tile_pool(name="scr", bufs=3) as scrp,
            tc.tile_pool(name="psum", bufs=4, space="PSUM") as psump,
        ):
            # Whole-shard resident SBUF tensors
            ua_sb = resident.tile([KA, B], dt_in, tag="ua")
            yp_sb = resident.tile([C, B], dt_in, tag="yp")
            UA_sb = resident.tile([KA, n_sh], dt_in, tag="UA")
            YT_sb = resident.tile([C, n_sh], dt_in, tag="YT")
            accA = resident.tile([128, n_iters], fp32, tag="accA")
            accMin = resident.tile([128, n_iters], fp32, tag="accMin")
            bias_m = resident.tile([128, 1], fp32, tag="biasm")
            nc.vector.memset(bias_m[:], M_MARGIN)

            nc.sync.dma_start(ua_sb[:], uaT_d[:])
            nc.sync.dma_start(yp_sb[:], ypT_d[:])
            nc.sync.dma_start(UA_sb[:], UA_d[:])
            nc.sync.dma_start(YT_sb[:], YT_d[:])

            for chunk in range(N_CHUNKS):
                cs = slice(chunk * 128, (chunk + 1) * 128)
                for t in range(n_tiles):
                    f = min(F_TILE, n_sh - t * F_TILE)
                    ns = slice(t * F_TILE, t * F_TILE + f)
                    idx = chunk * n_tiles + t

                    x1 = psump.tile([128, f], fp32, tag="x1")
                    nc.tensor.matmul(
                        x1[:], lhsT=ua_sb[:, cs], rhs=UA_sb[:, ns],
                        start=True, stop=False,
                    )
                    nc.tensor.matmul(
                        x1[:], lhsT=yp_sb[:, cs], rhs=YT_sb[:, ns],
                        start=False, stop=True,
                    )

                    scr = scrp.tile([128, F_TILE], fp32, tag="scr")
                    nc.scalar.activation(
                        scr[:, :f], x1[:],
                        mybir.ActivationFunctionType.Relu,
                        bias=bias_m[:], scale=-1.0,
                        accum_out=accA[:, idx : idx + 1],
                    )
                    nc.vector.tensor_reduce(
                        accMin[:, idx : idx + 1], x1[:],
                        axis=mybir.AxisListType.X, op=mybir.AluOpType.min,
                    )

            nc.sync.dma_start(accA_d[:], accA[:])
            nc.sync.dma_start(accMin_d[:], accMin[:])

    nc.finalize()
    return nc, n_tiles


def _prep_host(u, y, ind, U, Y):
    """Scatter + build device arrays + fp64 base sums."""
    u = np.asarray(u, dtype=np.float32)
    y = np.asarray(y, dtype=np.float32)
    ind = np.asarray(ind).astype(np.int64)
    U2 = np.array(U, dtype=np.float32, copy=True)
    Y2 = np.array(Y, dtype=np.float32, copy=True)
    U2[ind] = u
    Y2[ind] = y

    u64 = u.astype(np.float64)
    U64 = U2.astype(np.float64)
    u_sq64 = (u64 * u64).sum(axis=1)            # [B]
    U_sq64 = (U64 * U64).sum(axis=1)            # [N]
    # sum_{b,n} raw  (exact in fp64, factorized)
    s_raw = (
        N * u_sq64.sum()
        + B * U_sq64.sum()
        - 2.0 * (u64.sum(axis=0) @ U64.sum(axis=0))
    )

    # Augmented transposed operands (fp32 for now)
    uaT = np.empty((KA, B), dtype=np.float32)
    uaT[:BIT] = (-2.0 * u).T
    uaT[BIT] = 1.0
    uaT[BIT + 1] = u_sq64.astype(np.float32)
    UA = np.empty((KA, N), dtype=np.float32)
    UA[:BIT] = U2.T
    UA[BIT] = U_sq64.astype(np.float32)
    UA[BIT + 1] = 1.0

    ypT = np.ascontiguousarray((BIG * y).T)     # [C, B]
    YT = np.ascontiguousarray(Y2.T)             # [C, N]

    return u, y, U2, Y2, uaT, UA, ypT, YT, s_raw


_PROG_CACHE = {}


def _get_program():
    key = ("fp32", N_SH)
    if key not in _PROG_CACHE:
        _PROG_CACHE[key] = _build_program(N_SH, mybir.dt.float32)
    return _PROG_CACHE[key]


def kernel(u, y, ind, U, Y):
    u, y, U2, Y2, uaT, UA, ypT, YT, s_raw = _prep_host(u, y, ind, U, Y)

    nc, n_tiles = _get_program()
    in_maps = []
    for c in range(N_CORES):
        ns = slice(c * N_SH, (c + 1) * N_SH)
        in_maps.append({
            "uaT": uaT,
            "ypT": ypT,
            "UA": np.ascontiguousarray(UA[:, ns]),
            "YT": np.ascontiguousarray(YT[:, ns]),
        })

    res = run_bass_kernel_spmd(nc, in_maps, list(range(N_CORES)))
    results = res.results

    s_hinge = 0.0
    s_mraw = 0.0
    for c in range(N_CORES):
        accA = np.asarray(results[c]["accA"], dtype=np.float64)
        accMin = np.asarray(results[c]["accMin"], dtype=np.float64)
        s_hinge += accA.sum()

        # Detector: row-tiles that may contain match==0 pairs
        flagged = np.argwhere(accMin < DET_THRESH)
        for p, idx in flagged:
            chunk, t = divmod(idx, n_tiles)
            b = chunk * 128 + p
            n0 = c * N_SH + t * F_TILE
            n1 = min(n0 + F_TILE, c * N_SH + N_SH)
            match = y[b].astype(np.float64) @ Y2[n0:n1].astype(np.float64).T
            zcols = np.nonzero(match == 0.0)[0]
            for z in zcols:
                n_glob = n0 + z
                d = u[b].astype(np.float64) - U2[n_glob].astype(np.float64)
                s_mraw += float(d @ d)

    total2 = s_raw + s_hinge - s_mraw
    loss1 = 0.5 * total2 / (B * N)

    sign_u = np.sign(u)
    loss2 = ALPHA * np.abs(1.0 - sign_u).mean(dtype=np.float64)

    return np.array(loss1 + loss2, dtype=np.float32)


# revision 7
# speedup vs baseline: 2.3384x; 2.3384x over previous
"""DSH loss kernel for Trainium2 (8 NeuronCores, Bass/Tile).

Math (reference):
    U[ind] = u; Y[ind] = y
    raw[b,n]  = ||u_b||^2 - 2 u_b.U_n + ||U_n||^2          (>= 0 mathematically)
    dist      = max(raw, 0)
    match[b,n]= y_b . Y_n          (integer >= 0)
    m         = (match == 0)       ("mismatch" mask, statistically ~never 1)
    loss1 = mean( (1-m)*0.5*dist + m*0.5*relu(M - dist) )
    loss2 = ALPHA * mean(|1 - sign(u)|)

Decomposition:
    2*B*N*loss1 = S_raw + sum_{m=1} [ relu(M - raw) - raw ]
      S_raw factorizes: N*sum(u_sq) + B*sum(U_sq) - 2*colsum(u).colsum(U)
      -> computed exactly on host in fp64.
    The correction term needs the m==1 pairs. The device computes, for
    every (U-row, batch) pair, x1 = raw + BIG*match in one PSUM
    accumulation group (two bf16 matmuls, contraction dims 66 and 100),
    then one fused elementwise+reduce pass per tile:
        det = relu(T0 - x1)   with T0 chosen so  BIG >> T0 >> max(raw):
    x1 < T0 iff match == 0, so the per-row reduced det is nonzero iff
    that U-row has a match==0 pair -> exact detector. Flagged rows
    (normally none) are corrected exactly on host in fp64.
    The pass alternates between ScalarE (activation w/ accum_out) and
    VectorE (tensor_scalar subtract/min w/ accum_out) to balance engines.

Device tiling per core (shard = 12500 U/Y rows):
    98 tiles of 128 U-rows; stationary = UA/YT tile [K,128],
    moving = augmented uaT [66,512] / BIG*yT [100,512]; PSUM [128,512].
"""

import numpy as np
import ml_dtypes

import concourse.bass as bass
import concourse.mybir as mybir
import concourse.tile as tile
from concourse import bacc
from concourse.bass_utils import run_bass_kernel_spmd

# Problem constants (hardcoded per harness contract)
B = 512
BIT = 64
C = 100
N = 100000
N_CORES = 8
N_SH = N // N_CORES          # 12500
M_MARGIN = 2.0 * BIT         # 128.0
ALPHA = 0.1
BIG = 16384.0                # power of two; BIG*match exact in fp32/bf16
T0 = 8192.0                  # detector threshold: max(raw) << T0 << BIG
KA = BIT + 2                 # augmented contraction dim for the dist matmul
P_TILE = 128                 # U-rows per tile (PSUM partition dim)
F_B = B                      # moving free dim = full batch = 512
ACT_EVERY = 2                # tile t uses ScalarE if t % ACT_EVERY == 0 else DVE

BF16 = ml_dtypes.bfloat16


def _build_program(n_sh: int):
    fp32 = mybir.dt.float32
    bf16 = mybir.dt.bfloat16
    nc = bacc.Bacc("TRN2", target_bir_lowering=False)

    n_tiles = (n_sh + P_TILE - 1) // P_TILE

    uaT_d = nc.declare_dram_parameter("uaT", [KA, B], bf16, isOutput=False)
    ypT_d = nc.declare_dram_parameter("ypT", [C, B], bf16, isOutput=False)
    UA_d = nc.declare_dram_parameter("UA", [KA, n_sh], bf16, isOutput=False)
    YT_d = nc.declare_dram_parameter("YT", [C, n_sh], bf16, isOutput=False)
    accD_d = nc.declare_dram_parameter("accD", [128, n_tiles], fp32, isOutput=True)

    DMA_SLICE = 2048  # columns per input-streaming DMA

    with tile.TileContext(nc) as tc:
        with (
            tc.tile_pool(name="resident", bufs=1) as resident,
            tc.tile_pool(name="scr", bufs=4) as scrp,
            tc.tile_pool(name="psum", bufs=4, space="PSUM") as psump,
        ):
            ua_sb = resident.tile([KA, B], bf16, tag="ua")
            yp_sb = resident.tile([C, B], bf16, tag="yp")
            UA_sb = resident.tile([KA, n_sh], bf16, tag="UA")
            YT_sb = resident.tile([C, n_sh], bf16, tag="YT")
            accD = resident.tile([128, n_tiles], fp32, tag="accD")
            bias_t0 = resident.tile([128, 1], fp32, tag="biast0")

            nc.vector.memset(bias_t0[:], T0)
            nc.vector.memset(accD[:], 0.0)
            nc.sync.dma_start(ua_sb[:], uaT_d[:])
            nc.sync.dma_start(yp_sb[:], ypT_d[:])
            for s in range(0, n_sh, DMA_SLICE):
                w = min(DMA_SLICE, n_sh - s)
                nc.sync.dma_start(UA_sb[:, s : s + w], UA_d[:, s : s + w])
                nc.sync.dma_start(YT_sb[:, s : s + w], YT_d[:, s : s + w])

            for t in range(n_tiles):
                p = min(P_TILE, n_sh - t * P_TILE)
                ns = slice(t * P_TILE, t * P_TILE + p)

                x1 = psump.tile([P_TILE, F_B], fp32, tag="x1")
                nc.tensor.matmul(
                    x1[:p, :], lhsT=UA_sb[:, ns], rhs=ua_sb[:],
                    start=True, stop=False,
                )
                nc.tensor.matmul(
                    x1[:p, :], lhsT=YT_sb[:, ns], rhs=yp_sb[:],
                    start=False, stop=True,
                )

                col = accD[:p, t : t + 1]
                if t % ACT_EVERY == 0:
                    scr = scrp.tile([P_TILE, F_B], bf16, tag="scrA")
                    # relu(T0 - x1); accum col > 0 iff some match==0 here
                    nc.scalar.activation(
                        scr[:p, :], x1[:p, :],
                        mybir.ActivationFunctionType.Relu,
                        bias=bias_t0[:p, :], scale=-1.0,
                        accum_out=col,
                    )
                else:
                    scr = scrp.tile([P_TILE, F_B], bf16, tag="scrB")
                    # min(x1 - T0, 0); accum col < 0 iff some match==0 here
                    nc.vector.tensor_scalar(
                        scr[:p, :], x1[:p, :], T0, 0.0,
                        mybir.AluOpType.subtract, mybir.AluOpType.min,
                        accum_out=col,
                    )

            nc.sync.dma_start(accD_d[:], accD[:])

    nc.finalize()
    return nc, n_tiles


def _prep_host(u, y, ind, U, Y):
    """Scatter + device arrays (bf16) + fp64 base sum."""
    u = np.asarray(u, dtype=np.float32)
    y = np.asarray(y, dtype=np.float32)
    ind = np.asarray(ind).astype(np.int64)
    U2 = np.array(U, dtype=np.float32, copy=True)
    Y2 = np.array(Y, dtype=np.float32, copy=True)
    U2[ind] = u
    Y2[ind] = y

    u64 = u.astype(np.float64)
    U64 = U2.astype(np.float64)
    u_sq64 = (u64 * u64).sum(axis=1)            # [B]
    U_sq64 = (U64 * U64).sum(axis=1)            # [N]
    s_raw = (
        N * u_sq64.sum()
        + B * U_sq64.sum()
        - 2.0 * (u64.sum(axis=0) @ U64.sum(axis=0))
    )

    uaT = np.empty((KA, B), dtype=BF16)
    uaT[:BIT] = (-2.0 * u).T.astype(BF16)
    uaT[BIT] = BF16(1.0)
    uaT[BIT + 1] = u_sq64.astype(BF16)
    UA = np.empty((KA, N), dtype=BF16)
    UA[:BIT] = U2.T.astype(BF16)
    UA[BIT] = U_sq64.astype(BF16)
    UA[BIT + 1] = BF16(1.0)

    ypT = np.ascontiguousarray((BIG * y).T.astype(BF16))    # [C, B]
    YT = np.ascontiguousarray(Y2.T.astype(BF16))            # [C, N]

    return u, y, U2, Y2, uaT, UA, ypT, YT, s_raw


_PROG_CACHE = {}


def _get_program():
    key = ("v2", N_SH)
    if key not in _PROG_CACHE:
        _PROG_CACHE[key] = _build_program(N_SH)
    return _PROG_CACHE[key]


def kernel(u, y, ind, U, Y):
    u, y, U2, Y2, uaT, UA, ypT, YT, s_raw = _prep_host(u, y, ind, U, Y)

    nc, n_tiles = _get_program()
    in_maps = []
    for c in range(N_CORES):
        ns = slice(c * N_SH, (c + 1) * N_SH)
        in_maps.append({
            "uaT": uaT,
            "ypT": ypT,
            "UA": np.ascontiguousarray(UA[:, ns]),
            "YT": np.ascontiguousarray(YT[:, ns]),
        })

    res = run_bass_kernel_spmd(nc, in_maps, list(range(N_CORES)))
    results = res.results

    corr = 0.0
    for c in range(N_CORES):
        accD = np.asarray(results[c]["accD"], dtype=np.float64)
        flagged = np.argwhere(np.abs(accD) > 0.5)
        for p, t in flagged:
            n_glob = c * N_SH + t * P_TILE + p
            match = y.astype(np.float64) @ Y2[n_glob].astype(np.float64)  # [B]
            zrows = np.nonzero(match == 0.0)[0]
            for b in zrows:
                d = u[b].astype(np.float64) - U2[n_glob].astype(np.float64)
                raw = float(d @ d)
                corr += max(M_MARGIN - raw, 0.0) - raw

    total2 = s_raw + corr
    loss1 = 0.5 * total2 / (B * N)

    sign_u = np.sign(u)
    loss2 = ALPHA * np.abs(1.0 - sign_u).mean(dtype=np.float64)

    return np.array(loss1 + loss2, dtype=np.float32)


# revision 8
# speedup vs baseline: 2.3943x; 1.0239x over previous
"""DSH loss kernel for Trainium2 (8 NeuronCores, Bass/Tile).

Math (reference):
    U[ind] = u; Y[ind] = y
    raw[b,n]  = ||u_b||^2 - 2 u_b.U_n + ||U_n||^2          (>= 0 mathematically)
    dist      = max(raw, 0)
    match[b,n]= y_b . Y_n          (integer >= 0)
    m         = (match == 0)       ("mismatch" mask, statistically ~never 1)
    loss1 = mean( (1-m)*0.5*dist + m*0.5*relu(M - dist) )
    loss2 = ALPHA * mean(|1 - sign(u)|)

Decomposition:
    2*B*N*loss1 = S_raw + sum_{m=1} [ relu(M - raw) - raw ]
      S_raw factorizes: N*sum(u_sq) + B*sum(U_sq) - 2*colsum(u).colsum(U)
      -> computed exactly on host in fp64.
    The correction term needs the m==1 pairs. The device computes, for
    every (U-row, batch) pair, x1 = raw + BIG*match in one PSUM
    accumulation group (two bf16 matmuls, contraction dims 66 and 100),
    then one fused elementwise+reduce pass per tile:
        det = relu(T0 - x1)   with T0 chosen so  BIG >> T0 >> max(raw):
    x1 < T0 iff match == 0, so the per-row reduced det is nonzero iff
    that U-row has a match==0 pair -> exact detector. Flagged rows
    (normally none) are corrected exactly on host in fp64.
    The pass alternates between ScalarE (activation w/ accum_out) and
    VectorE (tensor_scalar subtract/min w/ accum_out) to balance engines.

Device tiling per core (shard = 12500 U/Y rows):
    98 tiles of 128 U-rows; stationary = UA/YT tile [K,128],
    moving = augmented uaT [66,512] / BIG*yT [100,512]; PSUM [128,512].
"""

import numpy as np
import ml_dtypes

import concourse.bass as bass
import concourse.mybir as mybir
import concourse.tile as tile
from concourse import bacc
from concourse.bass_utils import run_bass_kernel_spmd

# Problem constants (hardcoded per harness contract)
B = 512
BIT = 64
C = 100
N = 100000
N_CORES = 8
N_SH = N // N_CORES          # 12500
M_MARGIN = 2.0 * BIT         # 128.0
ALPHA = 0.1
BIG = 16384.0                # power of two; BIG*match exact in fp32/bf16
T0 = 8192.0                  # detector threshold: max(raw) << T0 << BIG
KA = BIT + 2                 # augmented contraction dim for the dist matmul
P_TILE = 128                 # U-rows per tile (PSUM partition dim)
F_B = B                      # moving free dim = full batch = 512
ACT_EVERY = 2                # tile t uses ScalarE if t % ACT_EVERY == 0 else DVE

BF16 = ml_dtypes.bfloat16


def _build_program(n_sh: int):
    fp32 = mybir.dt.float32
    bf16 = mybir.dt.bfloat16
    nc = bacc.Bacc("TRN2", target_bir_lowering=False)

    n_tiles = (n_sh + P_TILE - 1) // P_TILE

    uaT_d = nc.declare_dram_parameter("uaT", [KA, B], bf16, isOutput=False)
    ypT_d = nc.declare_dram_parameter("ypT", [C, B], bf16, isOutput=False)
    UA_d = nc.declare_dram_parameter("UA", [KA, n_sh], bf16, isOutput=False)
    YT_d = nc.declare_dram_parameter("YT", [C, n_sh], bf16, isOutput=False)
    accD_d = nc.declare_dram_parameter("accD", [128, n_tiles], fp32, isOutput=True)

    DMA_SLICE = 2048  # columns per input-streaming DMA

    with tile.TileContext(nc) as tc:
        with (
            tc.tile_pool(name="resident", bufs=1) as resident,
            tc.tile_pool(name="scr", bufs=4) as scrp,
            tc.tile_pool(name="psum", bufs=4, space="PSUM") as psump,
        ):
            ua_sb = resident.tile([KA, B], bf16, tag="ua")
            yp_sb = resident.tile([C, B], bf16, tag="yp")
            UA_sb = resident.tile([KA, n_sh], bf16, tag="UA")
            YT_sb = resident.tile([C, n_sh], bf16, tag="YT")
            accD = resident.tile([128, n_tiles], fp32, tag="accD")
            bias_t0 = resident.tile([128, 1], fp32, tag="biast0")

            # Moving operands first (tiny, needed by every matmul); then the
            # big gallery slices, small-first so tile 0 is ready ASAP.
            # UA on the sync queue, YT on the gpsimd queue -> parallel DMA.
            nc.sync.dma_start(ua_sb[:], uaT_d[:])
            nc.gpsimd.dma_start(yp_sb[:], ypT_d[:])
            s = 0
            for w in (512, 512, 1024, 2048, 4096):
                if s >= n_sh:
                    break
                w = min(w, n_sh - s)
                nc.sync.dma_start(UA_sb[:, s : s + w], UA_d[:, s : s + w])
                nc.gpsimd.dma_start(YT_sb[:, s : s + w], YT_d[:, s : s + w])
                s += w
            if s < n_sh:
                nc.sync.dma_start(UA_sb[:, s:], UA_d[:, s:])
                nc.gpsimd.dma_start(YT_sb[:, s:], YT_d[:, s:])
            nc.vector.memset(bias_t0[:], T0)
            nc.vector.memset(accD[:], 0.0)

            for t in range(n_tiles):
                p = min(P_TILE, n_sh - t * P_TILE)
                ns = slice(t * P_TILE, t * P_TILE + p)

                x1 = psump.tile([P_TILE, F_B], fp32, tag="x1")
                nc.tensor.matmul(
                    x1[:p, :], lhsT=UA_sb[:, ns], rhs=ua_sb[:],
                    start=True, stop=False,
                )
                nc.tensor.matmul(
                    x1[:p, :], lhsT=YT_sb[:, ns], rhs=yp_sb[:],
                    start=False, stop=True,
                )

                col = accD[:p, t : t + 1]
                if t % ACT_EVERY == 0:
                    scr = scrp.tile([P_TILE, F_B], bf16, tag="scrA")
                    # relu(T0 - x1); accum col > 0 iff some match==0 here
                    nc.scalar.activation(
                        scr[:p, :], x1[:p, :],
                        mybir.ActivationFunctionType.Relu,
                        bias=bias_t0[:p, :], scale=-1.0,
                        accum_out=col,
                    )
                else:
                    scr = scrp.tile([P_TILE, F_B], bf16, tag="scrB")
                    # min(x1 - T0, 0); accum col < 0 iff some match==0 here
                    nc.vector.tensor_scalar(
                        scr[:p, :], x1[:p, :], T0, 0.0,
                        mybir.AluOpType.subtract, mybir.AluOpType.min,
                        accum_out=col,
                    )

            nc.sync.dma_start(accD_d[:], accD[:])

    nc.finalize()
    return nc, n_tiles


def _prep_host(u, y, ind, U, Y):
    """Scatter + device arrays (bf16) + fp64 base sum."""
    u = np.asarray(u, dtype=np.float32)
    y = np.asarray(y, dtype=np.float32)
    ind = np.asarray(ind).astype(np.int64)
    U2 = np.array(U, dtype=np.float32, copy=True)
    Y2 = np.array(Y, dtype=np.float32, copy=True)
    U2[ind] = u
    Y2[ind] = y

    u64 = u.astype(np.float64)
    U64 = U2.astype(np.float64)
    u_sq64 = (u64 * u64).sum(axis=1)            # [B]
    U_sq64 = (U64 * U64).sum(axis=1)            # [N]
    s_raw = (
        N * u_sq64.sum()
        + B * U_sq64.sum()
        - 2.0 * (u64.sum(axis=0) @ U64.sum(axis=0))
    )

    uaT = np.empty((KA, B), dtype=BF16)
    uaT[:BIT] = (-2.0 * u).T.astype(BF16)
    uaT[BIT] = BF16(1.0)
    uaT[BIT + 1] = u_sq64.astype(BF16)
    UA = np.empty((KA, N), dtype=BF16)
    UA[:BIT] = U2.T.astype(BF16)
    UA[BIT] = U_sq64.astype(BF16)
    UA[BIT + 1] = BF16(1.0)

    ypT = np.ascontiguousarray((BIG * y).T.astype(BF16))    # [C, B]
    YT = np.ascontiguousarray(Y2.T.astype(BF16))            # [C, N]

    return u, y, U2, Y2, uaT, UA, ypT, YT, s_raw


_PROG_CACHE = {}


def _get_program():
    key = ("v2", N_SH)
    if key not in _PROG_CACHE:
        _PROG_CACHE[key] = _build_program(N_SH)
    return _PROG_CACHE[key]


def kernel(u, y, ind, U, Y):
    u, y, U2, Y2, uaT, UA, ypT, YT, s_raw = _prep_host(u, y, ind, U, Y)

    nc, n_tiles = _get_program()
    in_maps = []
    for c in range(N_CORES):
        ns = slice(c * N_SH, (c + 1) * N_SH)
        in_maps.append({
            "uaT": uaT,
            "ypT": ypT,
            "UA": np.ascontiguousarray(UA[:, ns]),
            "YT": np.ascontiguousarray(YT[:, ns]),
        })

    res = run_bass_kernel_spmd(nc, in_maps, list(range(N_CORES)))
    results = res.results

    corr = 0.0
    for c in range(N_CORES):
        accD = np.asarray(results[c]["accD"], dtype=np.float64)
        flagged = np.argwhere(np.abs(accD) > 0.5)
        for p, t in flagged:
            n_glob = c * N_SH + t * P_TILE + p
            match = y.astype(np.float64) @ Y2[n_glob].astype(np.float64)  # [B]
            zrows = np.nonzero(match == 0.0)[0]
            for b in zrows:
                d = u[b].astype(np.float64) - U2[n_glob].astype(np.float64)
                raw = float(d @ d)
                corr += max(M_MARGIN - raw, 0.0) - raw

    total2 = s_raw + corr
    loss1 = 0.5 * total2 / (B * N)

    sign_u = np.sign(u)
    loss2 = ALPHA * np.abs(1.0 - sign_u).mean(dtype=np.float64)

    return np.array(loss1 + loss2, dtype=np.float32)


# revision 13
# speedup vs baseline: 3.8045x; 1.5890x over previous
"""DSH loss kernel for Trainium2 (8 NeuronCores, Bass/Tile).

Math (reference):
    U[ind] = u; Y[ind] = y
    raw[b,n]  = ||u_b||^2 - 2 u_b.U_n + ||U_n||^2          (>= 0 mathematically)
    dist      = max(raw, 0)
    match[b,n]= y_b . Y_n          (integer >= 0)
    m         = (match == 0)       ("mismatch" mask, statistically ~never 1)
    loss1 = mean( (1-m)*0.5*dist + m*0.5*relu(M - dist) )
    loss2 = ALPHA * mean(|1 - sign(u)|)

Decomposition:
    2*B*N*loss1 = S_raw + sum_{m=1} [ relu(M - raw) - raw ]
      S_raw factorizes: N*sum(u_sq) + B*sum(U_sq) - 2*colsum(u).colsum(U)
      -> computed exactly on host in fp64.
    The correction term needs the m==1 pairs. The device computes, for
    every (U-row, batch) pair, x1 = raw + BIG*match in one PSUM
    accumulation group (two bf16 matmuls, contraction dims 66 and 100),
    then one fused elementwise+reduce pass per tile:
        det = relu(T0 - x1)   with T0 chosen so  BIG >> T0 >> max(raw):
    x1 < T0 iff match == 0, so the per-row reduced det is nonzero iff
    that U-row has a match==0 pair -> exact detector. Flagged rows
    (normally none) are corrected exactly on host in fp64.
    The pass alternates between ScalarE (activation w/ accum_out) and
    VectorE (tensor_scalar subtract/min w/ accum_out) to balance engines.

Device tiling per core (shard = 12500 U/Y rows):
    98 tiles of 128 U-rows; stationary = UA/YT tile [K,128],
    moving = augmented uaT [66,512] / BIG*yT [100,512]; PSUM [128,512].
"""

import numpy as np
import ml_dtypes

import concourse.bass as bass
import concourse.mybir as mybir
import concourse.tile as tile
from concourse import bacc
from concourse.bass_utils import run_bass_kernel_spmd

# Problem constants (hardcoded per harness contract)
B = 512
BIT = 64
C = 100
N = 100000
N_CORES = 8
N_SH = N // N_CORES          # 12500
M_MARGIN = 2.0 * BIT         # 128.0
ALPHA = 0.1
BIG = 16384.0                # power of two; BIG*match exact in fp32/bf16
T0 = 8192.0                  # detector threshold: max(raw) << T0 << BIG
KA = BIT + 2                 # augmented contraction dim for the dist matmul
P_TILE = 128                 # U-rows per tile (PSUM partition dim)
F_B = B                      # moving free dim = full batch = 512
ACT_EVERY = 2                # tile t uses ScalarE if t % ACT_EVERY == 0 else DVE

BF16 = ml_dtypes.bfloat16


def _build_program(n_sh: int):
    """v3: K=128 zero-padded operands (full-rate 216ns/MM), super-tiles of
    2x128 U-rows sharing one [128,1024] PSUM tile and ONE fused
    elementwise+accum pass, alternating ScalarE/VectorE."""
    fp32 = mybir.dt.float32
    bf16 = mybir.dt.bfloat16
    nc = bacc.Bacc("TRN2", target_bir_lowering=False)

    n_pad = ((n_sh + 2 * P_TILE - 1) // (2 * P_TILE)) * (2 * P_TILE)
    n_tiles = n_pad // P_TILE
    n_super = n_tiles // 2

    # all operands arrive zero-padded to K=128 rows (full matmul rate +
    # full-bandwidth 128-partition DMA)
    uaT_d = nc.declare_dram_parameter("uaT", [128, B], bf16, isOutput=False)
    ypT_d = nc.declare_dram_parameter("ypT", [128, B], bf16, isOutput=False)
    UA_d = nc.declare_dram_parameter("UA", [128, n_sh], bf16, isOutput=False)
    YT_d = nc.declare_dram_parameter("YT", [128, n_sh], bf16, isOutput=False)
    accD_d = nc.declare_dram_parameter("accD", [128, n_super], fp32, isOutput=True)

    with tile.TileContext(nc) as tc:
        with (
            tc.tile_pool(name="resident", bufs=1) as resident,
            tc.tile_pool(name="scr", bufs=4) as scrp,
            tc.tile_pool(name="psum", bufs=3, space="PSUM") as psump,
        ):
            ua_sb = resident.tile([128, B], bf16, tag="ua")
            yp_sb = resident.tile([128, B], bf16, tag="yp")
            UA_sb = resident.tile([128, n_pad], bf16, tag="UA")
            YT_sb = resident.tile([128, n_pad], bf16, tag="YT")
            accD = resident.tile([128, n_super], fp32, tag="accD")
            bias_t0 = resident.tile([128, 1], fp32, tag="biast0")

            # Moving operands first (tiny, needed by every matmul); then the
            # gallery slices, small-first so tile 0 is ready ASAP.
            # UA on the sync queue, YT on the gpsimd queue -> parallel DMA.
            nc.sync.dma_start(ua_sb[:], uaT_d[:])
            nc.gpsimd.dma_start(yp_sb[:], ypT_d[:])
            s = 0
            for w in (256, 256, 512, 1024, 2048, 4096):
                if s >= n_sh:
                    break
                w = min(w, n_sh - s)
                nc.sync.dma_start(UA_sb[:, s : s + w], UA_d[:, s : s + w])
                nc.gpsimd.dma_start(YT_sb[:, s : s + w], YT_d[:, s : s + w])
                s += w
            if s < n_sh:
                nc.sync.dma_start(UA_sb[:, s:n_sh], UA_d[:, s:])
                nc.gpsimd.dma_start(YT_sb[:, s:n_sh], YT_d[:, s:])

            # Column padding: UA pad cols = 0; YT pad cols = 1.0 so the
            # padded "gallery rows" match everything -> never flagged.
            if n_pad > n_sh:
                nc.vector.memset(UA_sb[:, n_sh:], 0.0)
                nc.vector.memset(YT_sb[:, n_sh:], 1.0)
            nc.vector.memset(bias_t0[:], T0)
            nc.vector.memset(accD[:], 0.0)

            for sidx in range(n_super):
                x1 = psump.tile([P_TILE, 1024], fp32, tag="x1")
                for h in (0, 1):
                    t = 2 * sidx + h
                    ns = slice(t * P_TILE, (t + 1) * P_TILE)
                    half = x1[:, h * 512 : (h + 1) * 512]
                    nc.tensor.matmul(
                        half, lhsT=UA_sb[:, ns], rhs=ua_sb[:],
                        start=True, stop=False,
                    )
                    nc.tensor.matmul(
                        half, lhsT=YT_sb[:, ns], rhs=yp_sb[:],
                        start=False, stop=True,
                    )

                col = accD[:, sidx : sidx + 1]
                if sidx % ACT_EVERY == 0:
                    scr = scrp.tile([P_TILE, 1024], bf16, tag="scrA")
                    # relu(T0 - x1); accum col > 0 iff some match==0 here
                    nc.scalar.activation(
                        scr[:], x1[:],
                        mybir.ActivationFunctionType.Relu,
                        bias=bias_t0[:], scale=-1.0,
                        accum_out=col,
                    )
                else:
                    scr = scrp.tile([P_TILE, 1024], bf16, tag="scrB")
                    # min(x1 - T0, 0); accum col < 0 iff some match==0 here
                    nc.vector.tensor_scalar(
                        scr[:], x1[:], T0, 0.0,
                        mybir.AluOpType.subtract, mybir.AluOpType.min,
                        accum_out=col,
                    )

            nc.sync.dma_start(accD_d[:], accD[:])

    nc.finalize()
    return nc, n_super


def _prep_host(u, y, ind, U, Y):
    """Scatter + device arrays (bf16) + fp64 base sum."""
    u = np.asarray(u, dtype=np.float32)
    y = np.asarray(y, dtype=np.float32)
    ind = np.asarray(ind).astype(np.int64)
    U2 = np.array(U, dtype=np.float32, copy=True)
    Y2 = np.array(Y, dtype=np.float32, copy=True)
    U2[ind] = u
    Y2[ind] = y

    u64 = u.astype(np.float64)
    U64 = U2.astype(np.float64)
    u_sq64 = (u64 * u64).sum(axis=1)            # [B]
    U_sq64 = (U64 * U64).sum(axis=1)            # [N]
    s_raw = (
        N * u_sq64.sum()
        + B * U_sq64.sum()
        - 2.0 * (u64.sum(axis=0) @ U64.sum(axis=0))
    )

    # K=128 zero-padded operands (rows: 64 dims | U_sq/1 | 1/u_sq | zeros)
    uaT = np.zeros((128, B), dtype=BF16)
    uaT[:BIT] = (-2.0 * u).T.astype(BF16)
    uaT[BIT] = BF16(1.0)
    uaT[BIT + 1] = u_sq64.astype(BF16)
    UA = np.zeros((128, N), dtype=BF16)
    UA[:BIT] = U2.T.astype(BF16)
    UA[BIT] = U_sq64.astype(BF16)
    UA[BIT + 1] = BF16(1.0)

    ypT = np.zeros((128, B), dtype=BF16)
    ypT[:C] = (BIG * y).T.astype(BF16)
    YT = np.zeros((128, N), dtype=BF16)
    YT[:C] = Y2.T.astype(BF16)

    return u, y, U2, Y2, uaT, UA, ypT, YT, s_raw


_PROG_CACHE = {}


def _get_program():
    key = ("v2", N_SH)
    if key not in _PROG_CACHE:
        _PROG_CACHE[key] = _build_program(N_SH)
    return _PROG_CACHE[key]


def kernel(u, y, ind, U, Y):
    u, y, U2, Y2, uaT, UA, ypT, YT, s_raw = _prep_host(u, y, ind, U, Y)

    nc, n_super = _get_program()
    in_maps = []
    for c in range(N_CORES):
        ns = slice(c * N_SH, (c + 1) * N_SH)
        in_maps.append({
            "uaT": uaT,
            "ypT": ypT,
            "UA": np.ascontiguousarray(UA[:, ns]),
            "YT": np.ascontiguousarray(YT[:, ns]),
        })

    res = run_bass_kernel_spmd(nc, in_maps, list(range(N_CORES)))
    results = res.results

    corr = 0.0
    for c in range(N_CORES):
        accD = np.asarray(results[c]["accD"], dtype=np.float64)
        flagged = np.argwhere(np.abs(accD) > 0.5)
        for p, sidx in flagged:
            # super-tile covers two 128-row tiles sharing partition p
            for h in (0, 1):
                n_loc = (2 * sidx + h) * P_TILE + p
                if n_loc >= N_SH:
                    continue  # padded column
                n_glob = c * N_SH + n_loc
                match = y.astype(np.float64) @ Y2[n_glob].astype(np.float64)
                zrows = np.nonzero(match == 0.0)[0]
                for b in zrows:
                    d = u[b].astype(np.float64) - U2[n_glob].astype(np.float64)
                    raw = float(d @ d)
                    corr += max(M_MARGIN - raw, 0.0) - raw

    total2 = s_raw + corr
    loss1 = 0.5 * total2 / (B * N)

    sign_u = np.sign(u)
    loss2 = ALPHA * np.abs(1.0 - sign_u).mean(dtype=np.float64)

    return np.array(loss1 + loss2, dtype=np.float32)


# revision 15
# speedup vs baseline: 3.9606x; 1.0410x over previous
"""DSH loss kernel for Trainium2 (8 NeuronCores, Bass/Tile).

Math (reference):
    U[ind] = u; Y[ind] = y
    raw[b,n]  = ||u_b||^2 - 2 u_b.U_n + ||U_n||^2          (>= 0 mathematically)
    dist      = max(raw, 0)
    match[b,n]= y_b . Y_n          (integer >= 0)
    m         = (match == 0)       ("mismatch" mask, statistically ~never 1)
    loss1 = mean( (1-m)*0.5*dist + m*0.5*relu(M - dist) )
    loss2 = ALPHA * mean(|1 - sign(u)|)

Decomposition:
    2*B*N*loss1 = S_raw + sum_{m=1} [ relu(M - raw) - raw ]
      S_raw factorizes: N*sum(u_sq) + B*sum(U_sq) - 2*colsum(u).colsum(U)
      -> computed exactly on host in fp64.
    The correction term needs the m==1 pairs. The device computes, for
    every (U-row, batch) pair, x1 = raw + BIG*match in one PSUM
    accumulation group (two bf16 matmuls, contraction dims 66 and 100),
    then one fused elementwise+reduce pass per tile:
        det = relu(T0 - x1)   with T0 chosen so  BIG >> T0 >> max(raw):
    x1 < T0 iff match == 0, so the per-row reduced det is nonzero iff
    that U-row has a match==0 pair -> exact detector. Flagged rows
    (normally none) are corrected exactly on host in fp64.
    The pass alternates between ScalarE (activation w/ accum_out) and
    VectorE (tensor_scalar subtract/min w/ accum_out) to balance engines.

Device tiling per core (shard = 12500 U/Y rows):
    98 tiles of 128 U-rows; stationary = UA/YT tile [K,128],
    moving = augmented uaT [66,512] / BIG*yT [100,512]; PSUM [128,512].
"""

import numpy as np
import ml_dtypes

import concourse.bass as bass
import concourse.mybir as mybir
import concourse.tile as tile
from concourse import bacc
from concourse.bass_utils import run_bass_kernel_spmd

# Problem constants (hardcoded per harness contract)
B = 512
BIT = 64
C = 100
N = 100000
N_CORES = 8
N_SH = N // N_CORES          # 12500
M_MARGIN = 2.0 * BIT         # 128.0
ALPHA = 0.1
BIG = 16384.0                # power of two; BIG*match exact in fp32/bf16
T0 = 8192.0                  # detector threshold: max(raw) << T0 << BIG
KA = BIT + 2                 # augmented contraction dim for the dist matmul
P_TILE = 128                 # U-rows per tile (PSUM partition dim)
F_B = B                      # moving free dim = full batch = 512
ACT_EVERY = 2                # tile t uses ScalarE if t % ACT_EVERY == 0 else DVE

BF16 = ml_dtypes.bfloat16


def _build_program(n_sh: int):
    """v3: K=128 zero-padded operands (full-rate 216ns/MM), super-tiles of
    2x128 U-rows sharing one [128,1024] PSUM tile and ONE fused
    elementwise+accum pass, alternating ScalarE/VectorE."""
    fp32 = mybir.dt.float32
    bf16 = mybir.dt.bfloat16
    nc = bacc.Bacc("TRN2", target_bir_lowering=False)

    n_pad = ((n_sh + 2 * P_TILE - 1) // (2 * P_TILE)) * (2 * P_TILE)
    n_tiles = n_pad // P_TILE
    n_super = n_tiles // 2

    # all operands arrive zero-padded to K=128 rows (full matmul rate +
    # full-bandwidth 128-partition DMA)
    uaT_d = nc.declare_dram_parameter("uaT", [128, B], bf16, isOutput=False)
    ypT_d = nc.declare_dram_parameter("ypT", [128, B], bf16, isOutput=False)
    UA_d = nc.declare_dram_parameter("UA", [128, n_sh], bf16, isOutput=False)
    YT_d = nc.declare_dram_parameter("YT", [128, n_sh], bf16, isOutput=False)
    accD_d = nc.declare_dram_parameter("accD", [128, n_super], fp32, isOutput=True)

    with tile.TileContext(nc) as tc:
        with (
            tc.tile_pool(name="resident", bufs=1) as resident,
            tc.tile_pool(name="scr", bufs=4) as scrp,
            tc.tile_pool(name="psum", bufs=4, space="PSUM") as psump,
        ):
            ua_sb = resident.tile([128, B], bf16, tag="ua")
            yp_sb = resident.tile([128, B], bf16, tag="yp")
            UA_sb = resident.tile([128, n_pad], bf16, tag="UA")
            YT_sb = resident.tile([128, n_pad], bf16, tag="YT")
            accD = resident.tile([128, n_super], fp32, tag="accD")
            bias_t0 = resident.tile([128, 1], fp32, tag="biast0")

            # Moving operands first (tiny, needed by every matmul); then the
            # gallery slices, small-first so tile 0 is ready ASAP.
            # UA on the sync queue, YT on the gpsimd queue -> parallel DMA.
            nc.sync.dma_start(ua_sb[:], uaT_d[:])
            nc.gpsimd.dma_start(yp_sb[:], ypT_d[:])
            s = 0
            widths = [256, 256, 512] + [1024] * 12
            for w in widths:
                if s >= n_sh:
                    break
                w = min(w, n_sh - s)
                nc.sync.dma_start(UA_sb[:, s : s + w], UA_d[:, s : s + w])
                nc.gpsimd.dma_start(YT_sb[:, s : s + w], YT_d[:, s : s + w])
                s += w
            if s < n_sh:
                nc.sync.dma_start(UA_sb[:, s:n_sh], UA_d[:, s:])
                nc.gpsimd.dma_start(YT_sb[:, s:n_sh], YT_d[:, s:])

            # Column padding: UA pad cols = 0; YT pad cols = 1.0 so the
            # padded "gallery rows" match everything -> never flagged.
            if n_pad > n_sh:
                nc.vector.memset(UA_sb[:, n_sh:], 0.0)
                nc.vector.memset(YT_sb[:, n_sh:], 1.0)
            nc.vector.memset(bias_t0[:], T0)
            nc.vector.memset(accD[:], 0.0)

            for sidx in range(n_super):
                x1 = psump.tile([P_TILE, 1024], fp32, tag="x1")
                for h in (0, 1):
                    t = 2 * sidx + h
                    ns = slice(t * P_TILE, (t + 1) * P_TILE)
                    half = x1[:, h * 512 : (h + 1) * 512]
                    nc.tensor.matmul(
                        half, lhsT=UA_sb[:, ns], rhs=ua_sb[:],
                        start=True, stop=False,
                    )
                    nc.tensor.matmul(
                        half, lhsT=YT_sb[:, ns], rhs=yp_sb[:],
                        start=False, stop=True,
                    )

                col = accD[:, sidx : sidx + 1]
                if sidx % ACT_EVERY == 0:
                    scr = scrp.tile([P_TILE, 1024], bf16, tag="scrA")
                    # relu(T0 - x1); accum col > 0 iff some match==0 here
                    nc.scalar.activation(
                        scr[:], x1[:],
                        mybir.ActivationFunctionType.Relu,
                        bias=bias_t0[:], scale=-1.0,
                        accum_out=col,
                    )
                else:
                    scr = scrp.tile([P_TILE, 1024], bf16, tag="scrB")
                    # min(x1 - T0, 0); accum col < 0 iff some match==0 here
                    nc.vector.tensor_scalar(
                        scr[:], x1[:], T0, 0.0,
                        mybir.AluOpType.subtract, mybir.AluOpType.min,
                        accum_out=col,
                    )

            nc.sync.dma_start(accD_d[:], accD[:])

    nc.finalize()
    return nc, n_super


def _prep_host(u, y, ind, U, Y):
    """Scatter + device arrays (bf16) + fp64 base sum."""
    u = np.asarray(u, dtype=np.float32)
    y = np.asarray(y, dtype=np.float32)
    ind = np.asarray(ind).astype(np.int64)
    U2 = np.array(U, dtype=np.float32, copy=True)
    Y2 = np.array(Y, dtype=np.float32, copy=True)
    U2[ind] = u
    Y2[ind] = y

    u64 = u.astype(np.float64)
    U64 = U2.astype(np.float64)
    u_sq64 = (u64 * u64).sum(axis=1)            # [B]
    U_sq64 = (U64 * U64).sum(axis=1)            # [N]
    s_raw = (
        N * u_sq64.sum()
        + B * U_sq64.sum()
        - 2.0 * (u64.sum(axis=0) @ U64.sum(axis=0))
    )

    # K=128 zero-padded operands (rows: 64 dims | U_sq/1 | 1/u_sq | zeros)
    uaT = np.zeros((128, B), dtype=BF16)
    uaT[:BIT] = (-2.0 * u).T.astype(BF16)
    uaT[BIT] = BF16(1.0)
    uaT[BIT + 1] = u_sq64.astype(BF16)
    UA = np.zeros((128, N), dtype=BF16)
    UA[:BIT] = U2.T.astype(BF16)
    UA[BIT] = U_sq64.astype(BF16)
    UA[BIT + 1] = BF16(1.0)

    ypT = np.zeros((128, B), dtype=BF16)
    ypT[:C] = (BIG * y).T.astype(BF16)
    YT = np.zeros((128, N), dtype=BF16)
    YT[:C] = Y2.T.astype(BF16)

    return u, y, U2, Y2, uaT, UA, ypT, YT, s_raw


_PROG_CACHE = {}


def _get_program():
    key = ("v2", N_SH)
    if key not in _PROG_CACHE:
        _PROG_CACHE[key] = _build_program(N_SH)
    return _PROG_CACHE[key]


def kernel(u, y, ind, U, Y):
    u, y, U2, Y2, uaT, UA, ypT, YT, s_raw = _prep_host(u, y, ind, U, Y)

    nc, n_super = _get_program()
    in_maps = []
    for c in range(N_CORES):
        ns = slice(c * N_SH, (c + 1) * N_SH)
        in_maps.append({
            "uaT": uaT,
            "ypT": ypT,
            "UA": np.ascontiguousarray(UA[:, ns]),
            "YT": np.ascontiguousarray(YT[:, ns]),
        })

    res = run_bass_kernel_spmd(nc, in_maps, list(range(N_CORES)))
    results = res.results

    corr = 0.0
    for c in range(N_CORES):
        accD = np.asarray(results[c]["accD"], dtype=np.float64)
        flagged = np.argwhere(np.abs(accD) > 0.5)
        for p, sidx in flagged:
            # super-tile covers two 128-row tiles sharing partition p
            for h in (0, 1):
                n_loc = (2 * sidx + h) * P_TILE + p
                if n_loc >= N_SH:
                    continue  # padded column
                n_glob = c * N_SH + n_loc
                match = y.astype(np.float64) @ Y2[n_glob].astype(np.float64)
                zrows = np.nonzero(match == 0.0)[0]
                for b in zrows:
                    d = u[b].astype(np.float64) - U2[n_glob].astype(np.float64)
                    raw = float(d @ d)
                    corr += max(M_MARGIN - raw, 0.0) - raw

    total2 = s_raw + corr
    loss1 = 0.5 * total2 / (B * N)

    sign_u = np.sign(u)
    loss2 = ALPHA * np.abs(1.0 - sign_u).mean(dtype=np.float64)

    return np.array(loss1 + loss2, dtype=np.float32)
